# revision 9
# baseline (speedup 1.0000x reference)
"""Trainium2 Bass kernel for an 8-layer dense MLP (784->512x6->10) + softmax.

Strategy (hardcoded for batch=65536, 8 NeuronCores, pure data parallel):
  - Each core handles 8192 rows of the batch; weights replicated.
  - Dropout masks (jax threefry, key 42) are bit-exactly precomputed on host
    CPU and shipped as {0,1} uint8 masks; the 1/(1-p) rescale is folded into
    the next layer's weights on host.
  - On-chip, activations are kept feature-major ([feature, batch] = h^T) so
    every hidden layer is a chain of 128x128 fp32r matmuls with the batch
    tile (512) as the moving free dim — no transposes anywhere (x is
    transposed on host).
  - The last layer (512->10) flips to batch-major by using h^T as lhsT, so
    softmax reduces along the free dim.
"""

import numpy as np

BATCH = 65536
D_IN = 784
KO1 = 7                   # 896 = 7*128 padded input-feature chunks
D_PAD = KO1 * 128
H = 512
KO = H // 128             # 4 feature chunks for hidden layers
C = 10
N_CORES = 8
B_CORE = BATCH // N_CORES  # 8192
BT = 512                   # batch tile (matmul moving free dim)

DROP_LAYERS = (2, 4, 6)    # dropout applied to these layers' outputs
KEEP = {2: 0.8, 4: 0.7, 6: 0.5}


def build_bass(b_core: int):
    """Build the Bass module for one core processing b_core batch rows."""
    import concourse.mybir as mybir
    import concourse.tile as tile
    from concourse import bacc

    f32 = mybir.dt.float32
    f32r = mybir.dt.float32r
    u8 = mybir.dt.uint8
    AF = mybir.ActivationFunctionType
    ALU = mybir.AluOpType
    AX = mybir.AxisListType

    nbt = b_core // BT

    nc = bacc.Bacc("TRN2", target_bir_lowering=False, debug=False)

    xT = nc.dram_tensor("xT", [D_PAD, b_core], f32r, kind="ExternalInput")
    w_h = {1: nc.dram_tensor("w1", [D_PAD, H], f32r, kind="ExternalInput")}
    for l in range(2, 8):
        w_h[l] = nc.dram_tensor(f"w{l}", [H, H], f32r, kind="ExternalInput")
    w8_h = nc.dram_tensor("w8", [H, C], f32r, kind="ExternalInput")
    bias17_h = nc.dram_tensor("bias17", [128, 28], f32, kind="ExternalInput")
    b8r_h = nc.dram_tensor("b8r", [128, 4 * C], f32, kind="ExternalInput")
    m_h = {
        l: nc.dram_tensor(f"m{l}", [H, b_core], u8, kind="ExternalInput")
        for l in DROP_LAYERS
    }
    y_h = nc.dram_tensor("y", [b_core, C], f32, kind="ExternalOutput")

    with tile.TileContext(nc) as tc:
        with (
            tc.tile_pool(name="wpool", bufs=1) as wpool,
            tc.tile_pool(name="xpool", bufs=3) as xpool,
            tc.tile_pool(name="hpool", bufs=3) as hpool,
            tc.tile_pool(name="mpool", bufs=2) as mpool,
            tc.tile_pool(name="spool", bufs=3) as spool,
            tc.tile_pool(name="opool", bufs=3) as opool,
            tc.tile_pool(name="psum", bufs=4, space="PSUM") as pp,
            tc.tile_pool(name="psum8", bufs=2, space="PSUM") as pp8,
            tc.tile_pool(name="psumw", bufs=1, space="PSUM") as ppw,
        ):
            xT_r = xT.ap().rearrange("(ko p) b -> p ko b", p=128)
            m_r = {l: m_h[l].ap().rearrange("(ko p) b -> p ko b", p=128) for l in DROP_LAYERS}
            y_r = y_h.ap().rearrange("(q p) c -> p q c", p=128)

            def load_bt(bt):
                bs = bt * BT
                xt = xpool.tile([128, KO1, BT], f32r, tag="xt", name="xt")
                nc.sync.dma_start(xt[:], xT_r[:, :, bs : bs + BT])
                mt = {}
                for l in DROP_LAYERS:
                    mt[l] = mpool.tile([128, KO, BT], u8, tag=f"m{l}", name=f"m{l}_t")
                    nc.gpsimd.dma_start(mt[l][:], m_r[l][:, :, bs : bs + BT])
                return xt, mt

            # Warm the PE HAM clock-gate with dummy matmuls that run during
            # the initial DMA wait (~3.4us of activity flips K=4/8 -> 8/8).
            warm_w = wpool.tile([128, 128], f32, tag="warm_w")
            warm_x = wpool.tile([128, BT], f32, tag="warm_x")
            nc.vector.memset(warm_w[:], 0)
            nc.vector.memset(warm_x[:], 0)
            warm_ps = ppw.tile([128, BT], mybir.dt.float32, tag="warm_ps")
            for _ in range(8):
                nc.tensor.matmul(warm_ps[:], lhsT=warm_w[:], rhs=warm_x[:])

            # First batch-tile loads go out before the weight DMAs so layer-1
            # compute can start after ~4MB instead of after all weights.
            xt0, mt0 = load_bt(0)

            # --- resident weights/biases (stream in behind the first tile) ---
            w1_t = wpool.tile([128, KO1, H], f32r, tag="w1")
            nc.sync.dma_start(w1_t[:], w_h[1].ap().rearrange("(ko p) n -> p ko n", p=128))
            w_t = {1: w1_t}
            for l in range(2, 8):
                w_t[l] = wpool.tile([128, KO, H], f32r, tag=f"w{l}", name=f"w{l}_t")
                nc.sync.dma_start(
                    w_t[l][:], w_h[l].ap().rearrange("(ko p) n -> p ko n", p=128)
                )
            w8_t = wpool.tile([128, KO, C], f32r, tag="w8")
            nc.sync.dma_start(w8_t[:], w8_h.ap().rearrange("(ko p) c -> p ko c", p=128))
            bias17_t = wpool.tile([128, 28], f32, tag="bias17")
            nc.sync.dma_start(bias17_t[:], bias17_h.ap())
            b8r_t = wpool.tile([128, 4 * C], f32, tag="b8r")
            nc.sync.dma_start(b8r_t[:], b8r_h.ap())

            for bt in range(nbt):
                xt, mt = (xt0, mt0) if bt == 0 else load_bt(bt)

                # --- hidden layers 1..7, feature-major h^T [128, KO, BT] ---
                h = None
                for l in range(1, 8):
                    ko_in = KO1 if l == 1 else KO
                    src = xt if l == 1 else h
                    hn = hpool.tile([128, KO, BT], f32r, tag="h")
                    for n in range(KO):
                        ps = pp.tile([128, BT], mybir.dt.float32, tag="ps")
                        for k in range(ko_in):
                            nc.tensor.matmul(
                                ps[:],
                                lhsT=w_t[l][:, k, n * 128 : (n + 1) * 128],
                                rhs=src[:, k, :],
                                start=(k == 0),
                                stop=(k == ko_in - 1),
                            )
                        # relu(psum + bias) fused, PSUM -> SBUF
                        nc.scalar.activation(
                            hn[:, n, :],
                            ps[:],
                            AF.Relu,
                            bias=bias17_t[:, (l - 1) * 4 + n : (l - 1) * 4 + n + 1],
                        )
                        if l in DROP_LAYERS:
                            nc.vector.tensor_tensor(
                                hn[:, n, :], hn[:, n, :], mt[l][:, n, :], ALU.mult
                            )
                    h = hn

                # --- layer 8 (512->10), batch-major out [128, 4, 10] ---
                ps8 = pp8.tile([128, 4, C], mybir.dt.float32, tag="ps8")
                for m in range(4):
                    for k in range(KO):
                        nc.tensor.matmul(
                            ps8[:, m, :],
                            lhsT=h[:, k, m * 128 : (m + 1) * 128],
                            rhs=w8_t[:, k, :],
                            start=(k == 0),
                            stop=(k == KO - 1),
                        )
                # logits = psum + b8 (b8 replicated across partitions on host)
                lt = spool.tile([128, 4, C], f32, tag="lt")
                nc.vector.tensor_tensor(lt[:], ps8[:], b8r_t[:], ALU.add)
                # stable softmax along free dim (10 classes)
                negmax = spool.tile([128, 4], f32, tag="negmax")
                nc.vector.tensor_reduce(negmax[:], lt[:], AX.X, ALU.max, negate=True)
                ex = spool.tile([128, 4, C], f32, tag="ex")
                sums = spool.tile([128, 4], f32, tag="sums")
                for m in range(4):
                    nc.scalar.activation(
                        ex[:, m, :],
                        lt[:, m, :],
                        AF.Exp,
                        bias=negmax[:, m : m + 1],
                        accum_out=sums[:, m : m + 1],
                    )
                rec = spool.tile([128, 4], f32, tag="rec")
                nc.vector.reciprocal(rec[:], sums[:])
                ot = opool.tile([128, 4, C], f32, tag="ot")
                for m in range(4):
                    nc.vector.tensor_scalar_mul(ot[:, m, :], ex[:, m, :], rec[:, m : m + 1])
                nc.gpsimd.dma_start(y_r[:, bt * 4 : (bt + 1) * 4, :], ot[:])

    nc.compile()
    return nc


def host_prepare(inputs: dict) -> tuple[dict, dict]:
    """Fold dropout scaling into weights, compute masks, transpose/shard x.

    Returns (shared_inputs, per_core_varying) where per_core_varying maps
    name -> list of 8 per-core arrays.
    """
    import jax

    x = np.asarray(inputs["x"], dtype=np.float32)
    W = {i: np.asarray(inputs[f"W{i}"], dtype=np.float32) for i in range(1, 9)}
    b = {i: np.asarray(inputs[f"b{i}"], dtype=np.float32) for i in range(1, 9)}

    # Dropout masks — bit-exact replication of the reference's PRNG stream.
    cpu = jax.devices("cpu")[0]
    with jax.default_device(cpu):
        dk = jax.random.split(jax.random.key(42), 3)
        keeps = {
            l: np.asarray(
                jax.random.bernoulli(dk[i], KEEP[l], (BATCH, H)), dtype=np.uint8
            )
            for i, l in enumerate(DROP_LAYERS)
        }

    # Fold 1/(1-p) into the next layer's weights.
    Wf = dict(W)
    for l in DROP_LAYERS:
        Wf[l + 1] = (W[l + 1] / np.float32(KEEP[l])).astype(np.float32)

    # Pad layer 1 to 896 input features.
    W1p = np.zeros((D_PAD, H), dtype=np.float32)
    W1p[:D_IN] = Wf[1]

    xTp = np.zeros((D_PAD, BATCH), dtype=np.float32)
    xTp[:D_IN] = x.T

    bias17 = np.empty((128, 28), dtype=np.float32)
    for l in range(1, 8):
        bias17[:, (l - 1) * 4 : l * 4] = b[l].reshape(4, 128).T
    b8r = np.tile(b[8], (128, 4)).astype(np.float32)

    shared = {
        "w1": np.ascontiguousarray(W1p),
        "w8": np.ascontiguousarray(Wf[8]),
        "bias17": bias17,
        "b8r": b8r,
    }
    for l in range(2, 8):
        shared[f"w{l}"] = np.ascontiguousarray(Wf[l])

    per_core = {"xT": [], "m2": [], "m4": [], "m6": []}
    mT = {l: keeps[l].T for l in DROP_LAYERS}
    for c in range(N_CORES):
        sl = slice(c * B_CORE, (c + 1) * B_CORE)
        per_core["xT"].append(np.ascontiguousarray(xTp[:, sl]))
        for l in DROP_LAYERS:
            per_core[f"m{l}"].append(np.ascontiguousarray(mT[l][:, sl]))
    return shared, per_core


def run_hw(inputs: dict, trace: bool = False):
    from concourse import bass_utils

    shared, per_core = host_prepare(inputs)
    nc = build_bass(B_CORE)
    in_maps = [
        {**shared, **{k: v[c] for k, v in per_core.items()}} for c in range(N_CORES)
    ]
    res = bass_utils.run_bass_kernel_spmd(
        nc, in_maps, core_ids=list(range(N_CORES)), trace=trace
    )
    out = np.concatenate([r["y"] for r in res.results], axis=0)
    return out.astype(np.float32), res


def kernel(**inputs) -> np.ndarray:
    return run_hw(inputs, trace=False)[0]
